# revision 6
# baseline (speedup 1.0000x reference)
"""Trainium2 Bass kernel for nn_EstimatorCV (segment_reduce, 8 NeuronCores).

Data-parallel over N: each of the 8 cores reads a 16384-row shard of
`features`, computes per-class partial sums (sum / sumsq / count) via
one-hot matmuls on the TensorEngine, the [C, 2*(A+1)] partials are
all-reduced across the 8 cores, and every core redundantly computes the
tiny EMA combine and writes the [C,A] outputs.

Host-side prep per core:
  - append a ones column to features ([16384, 257]) so the count falls
    out of the same matmuls (column 256 of the partials),
  - reorder rows so each 1 MiB DMA lands contiguous per SBUF partition,
  - transpose labels to [128 partitions, 128 tiles] float32.

The device work runs in a subprocess with a clean JAX environment so the
caller's JAX state (e.g. JAX_PLATFORMS=cpu) cannot break the PJRT path.
"""

import os
import subprocess
import sys
import tempfile

import numpy as np

N_CORES = 8
N, A, C = 131072, 256, 19
RPC = N // N_CORES  # rows per core = 16384
P = 128  # SBUF partitions / rows per matmul tile
AP1 = A + 1  # 257: features + ones column
SUB = 8  # row-tiles per DMA group
GROUPS = RPC // (P * SUB)  # 16 groups of ~1MiB per core
GCOLS = SUB * AP1  # 2056 f32 per partition per group
N_TILES = GROUPS * SUB  # 128 row-tiles per core


def _prep_core_inputs(features, labels, cov, ave, amt, core):
    """Build the in_map for one core from the full inputs."""
    sh = features[core * RPC : (core + 1) * RPC]
    f = np.empty((RPC, AP1), np.float32)
    f[:, :A] = sh
    f[:, A] = 1.0
    # [RPC, AP1] -> [GROUPS, P, SUB*AP1] with rows interleaved so that
    # group g, partition p, subtile j holds row g*P*SUB + j*P + p.
    f = (
        f.reshape(GROUPS, SUB, P, AP1)
        .transpose(0, 2, 1, 3)
        .reshape(GROUPS, P, GCOLS)
    )
    lt = (
        labels[core * RPC : (core + 1) * RPC]
        .reshape(N_TILES, P)
        .T.astype(np.float32)
    )
    return {
        "features": np.ascontiguousarray(f),
        "labels": np.ascontiguousarray(lt),
        "cov_in": cov,
        "ave_in": ave,
        "amount_in": amt,
    }


def _build():
    """Construct the Bass/Tile program (one SPMD NeuronCore view)."""
    from contextlib import ExitStack

    import concourse.mybir as mybir
    import concourse.tile as tile
    from concourse import bacc

    f32 = mybir.dt.float32
    nc = bacc.Bacc(trn_type="TRN2", num_devices=N_CORES)

    feats = nc.dram_tensor("features", [GROUPS, P, GCOLS], f32, kind="ExternalInput")
    labels = nc.dram_tensor("labels", [P, N_TILES], f32, kind="ExternalInput")
    cov_in = nc.dram_tensor("cov_in", [C, A], f32, kind="ExternalInput")
    ave_in = nc.dram_tensor("ave_in", [C, A], f32, kind="ExternalInput")
    amount_in = nc.dram_tensor("amount_in", [C], f32, kind="ExternalInput")
    cov_out = nc.dram_tensor("cov_out", [C, A], f32, kind="ExternalOutput")
    ave_out = nc.dram_tensor("ave_out", [C, A], f32, kind="ExternalOutput")
    amount_out = nc.dram_tensor("amount_out", [C], f32, kind="ExternalOutput")
    # Collective bounce buffers (internal DRAM; output must be Shared).
    cc_in = nc.dram_tensor("cc_in", [C, 2 * AP1], f32)
    cc_out = nc.dram_tensor("cc_out", [C, 2 * AP1], f32, addr_space="Shared")

    eq = mybir.AluOpType.is_equal
    mult = mybir.AluOpType.mult
    add = mybir.AluOpType.add

    with tile.TileContext(nc) as tc, ExitStack() as ctx:
        const_pool = ctx.enter_context(tc.tile_pool(name="const", bufs=1))
        feat_pool = ctx.enter_context(tc.tile_pool(name="feat", bufs=3))
        sq_pool = ctx.enter_context(tc.tile_pool(name="sq", bufs=2))
        oh_pool = ctx.enter_context(tc.tile_pool(name="oh", bufs=4))
        psum_pool = ctx.enter_context(tc.tile_pool(name="psum", bufs=1, space="PSUM"))
        tail_pool = ctx.enter_context(tc.tile_pool(name="tail", bufs=1))

        # --- constants / small loads ---
        iota_i = const_pool.tile([P, C], mybir.dt.int32)
        nc.gpsimd.iota(iota_i[:], pattern=[[1, C]], base=0, channel_multiplier=0)
        iota_f = const_pool.tile([P, C], f32)
        nc.vector.tensor_copy(iota_f[:], iota_i[:])

        labels_t = const_pool.tile([P, N_TILES], f32)
        nc.sync.dma_start(labels_t[:], labels.ap())

        cov_t = tail_pool.tile([C, A], f32)
        nc.sync.dma_start(cov_t[:], cov_in.ap())
        avein_t = tail_pool.tile([C, A], f32)
        nc.sync.dma_start(avein_t[:], ave_in.ap())
        amount_t = tail_pool.tile([C, 1], f32)
        nc.sync.dma_start(amount_t[:], amount_in.ap().unsqueeze(1))

        # --- main loop: per-class partial sums into PSUM ---
        psum_s = psum_pool.tile([C, AP1], f32)  # [sum(x) | count]
        psum_q = psum_pool.tile([C, AP1], f32)  # [sum(x^2) | count]

        for g in range(GROUPS):
            feat = feat_pool.tile([P, GCOLS], f32)
            nc.sync.dma_start(feat[:], feats.ap()[g])
            sq = sq_pool.tile([P, GCOLS], f32)
            nc.scalar.square(sq[:], feat[:])
            for j in range(SUB):
                it = g * SUB + j
                oh = oh_pool.tile([P, C], f32)
                nc.vector.tensor_tensor(
                    out=oh[:],
                    in0=iota_f[:],
                    in1=labels_t[:, it : it + 1].to_broadcast([P, C]),
                    op=eq,
                )
                first = it == 0
                last = it == N_TILES - 1
                nc.tensor.matmul(
                    psum_s[:],
                    lhsT=oh[:],
                    rhs=feat[:, j * AP1 : (j + 1) * AP1],
                    start=first,
                    stop=last,
                )
                nc.tensor.matmul(
                    psum_q[:],
                    lhsT=oh[:],
                    rhs=sq[:, j * AP1 : (j + 1) * AP1],
                    start=first,
                    stop=last,
                )

        # --- all-reduce the partials across the 8 cores ---
        part = tail_pool.tile([C, 2 * AP1], f32)
        nc.vector.tensor_copy(part[:, 0:AP1], psum_s[:])
        nc.vector.tensor_copy(part[:, AP1 : 2 * AP1], psum_q[:])
        nc.sync.dma_start(cc_in.ap(), part[:])
        nc.gpsimd.collective_compute(
            "AllReduce",
            add,
            replica_groups=[list(range(N_CORES))],
            ins=[cc_in.ap()],
            outs=[cc_out.ap()],
        )
        red = tail_pool.tile([C, 2 * AP1], f32)
        nc.sync.dma_start(red[:], cc_out.ap())

        # --- EMA combine (tiny, replicated on every core) ---
        s_ap = red[:, 0:A]
        cnt = red[:, A : A + 1]
        sq_ap = red[:, AP1 : AP1 + A]

        cntc = tail_pool.tile([C, 1], f32)
        nc.vector.tensor_scalar_max(cntc[:], cnt, 1.0)
        inv = tail_pool.tile([C, 1], f32)
        nc.vector.reciprocal(inv[:], cntc[:])

        ave = tail_pool.tile([C, A], f32)
        nc.vector.tensor_tensor(
            out=ave[:], in0=s_ap, in1=inv[:, 0:1].to_broadcast([C, A]), op=mult
        )

        # var = (sq - 2*ave*s + cnt*ave^2) / cntc
        t1 = tail_pool.tile([C, A], f32)
        nc.vector.tensor_tensor(out=t1[:], in0=ave[:], in1=s_ap, op=mult)
        nc.vector.tensor_scalar_mul(t1[:], t1[:], -2.0)
        t2 = tail_pool.tile([C, A], f32)
        nc.vector.tensor_tensor(out=t2[:], in0=ave[:], in1=ave[:], op=mult)
        nc.vector.tensor_tensor(
            out=t2[:], in0=t2[:], in1=cnt.to_broadcast([C, A]), op=mult
        )
        var = tail_pool.tile([C, A], f32)
        nc.vector.tensor_add(var[:], sq_ap, t1[:])
        nc.vector.tensor_add(var[:], var[:], t2[:])
        nc.vector.tensor_tensor(
            out=var[:], in0=var[:], in1=inv[:, 0:1].to_broadcast([C, A]), op=mult
        )

        # w = cnt / max(cnt + Amount, tiny); w1 = 1 - w
        den = tail_pool.tile([C, 1], f32)
        nc.vector.tensor_tensor(out=den[:], in0=cnt, in1=amount_t[:], op=add)
        nc.vector.tensor_scalar_max(den[:], den[:], 1e-30)
        invd = tail_pool.tile([C, 1], f32)
        nc.vector.reciprocal(invd[:], den[:])
        w = tail_pool.tile([C, 1], f32)
        nc.vector.tensor_tensor(out=w[:], in0=cnt, in1=invd[:], op=mult)
        w1 = tail_pool.tile([C, 1], f32)
        nc.vector.tensor_scalar(
            out=w1[:], in0=w[:], scalar1=-1.0, scalar2=1.0, op0=mult, op1=add
        )
        ww1 = tail_pool.tile([C, 1], f32)
        nc.vector.tensor_tensor(out=ww1[:], in0=w[:], in1=w1[:], op=mult)

        # additional = w*(1-w)*(Ave - ave)^2
        d = tail_pool.tile([C, A], f32)
        nc.vector.tensor_sub(d[:], avein_t[:], ave[:])
        nc.vector.tensor_tensor(out=d[:], in0=d[:], in1=d[:], op=mult)
        nc.vector.tensor_tensor(
            out=d[:], in0=d[:], in1=ww1[:, 0:1].to_broadcast([C, A]), op=mult
        )

        w_b = w[:, 0:1].to_broadcast([C, A])
        w1_b = w1[:, 0:1].to_broadcast([C, A])

        # cov_new = CoVariance*w1 + var*w + additional
        covn = tail_pool.tile([C, A], f32)
        nc.vector.tensor_tensor(out=covn[:], in0=cov_t[:], in1=w1_b, op=mult)
        nc.vector.tensor_tensor(out=var[:], in0=var[:], in1=w_b, op=mult)
        nc.vector.tensor_add(covn[:], covn[:], var[:])
        nc.vector.tensor_add(covn[:], covn[:], d[:])

        # ave_new = Ave*w1 + ave*w
        aven = tail_pool.tile([C, A], f32)
        nc.vector.tensor_tensor(out=aven[:], in0=avein_t[:], in1=w1_b, op=mult)
        nc.vector.tensor_tensor(out=ave[:], in0=ave[:], in1=w_b, op=mult)
        nc.vector.tensor_add(aven[:], aven[:], ave[:])

        # amount_new = Amount + cnt
        amn = tail_pool.tile([C, 1], f32)
        nc.vector.tensor_tensor(out=amn[:], in0=amount_t[:], in1=cnt, op=add)

        nc.sync.dma_start(cov_out.ap(), covn[:])
        nc.sync.dma_start(ave_out.ap(), aven[:])
        nc.sync.dma_start(amount_out.ap().unsqueeze(1), amn[:])

    nc.compile()
    return nc


def _run_on_device(features, labels, cov, ave, amt, trace=False, tmpdir=None):
    """Shard inputs, compile + execute on 8 NeuronCores, return outputs.

    Must run in an interpreter whose JAX sees the axon NeuronCore devices.
    Returns (cov_new, ave_new, amount_new, exec_time_ns_or_None).
    """
    from concourse.bass_utils import run_bass_kernel_spmd

    nc = _build()
    in_maps = [
        _prep_core_inputs(features, labels, cov, ave, amt, c)
        for c in range(N_CORES)
    ]
    res = run_bass_kernel_spmd(
        nc,
        in_maps,
        list(range(N_CORES)),
        trace=trace,
        tmpdir=tmpdir,
    )
    r = res.results[0]
    return r["cov_out"], r["ave_out"], r["amount_out"], res.exec_time_ns


def _worker_main(argv):
    in_path, out_path = argv[0], argv[1]
    trace = "--trace" in argv
    dat = np.load(in_path)
    cov, ave, amt, exec_ns = _run_on_device(
        dat["features"],
        dat["labels"],
        dat["cov"],
        dat["ave"],
        dat["amt"],
        trace=trace,
        tmpdir=(argv[argv.index("--trace") + 1] if trace else None),
    )
    np.savez(
        out_path,
        cov=cov,
        ave=ave,
        amt=amt,
        exec_ns=np.int64(exec_ns if exec_ns is not None else -1),
    )


def kernel(features, labels, CoVariance, Ave, Amount):
    features = np.ascontiguousarray(np.asarray(features), dtype=np.float32)
    labels = np.ascontiguousarray(np.asarray(labels), dtype=np.int64)
    cov = np.ascontiguousarray(np.asarray(CoVariance), dtype=np.float32)
    ave = np.ascontiguousarray(np.asarray(Ave), dtype=np.float32)
    amt = np.ascontiguousarray(np.asarray(Amount), dtype=np.float32)

    with tempfile.TemporaryDirectory() as td:
        in_path = os.path.join(td, "in.npz")
        out_path = os.path.join(td, "out.npz")
        np.savez(in_path, features=features, labels=labels, cov=cov, ave=ave, amt=amt)
        env = dict(os.environ)
        env.pop("JAX_PLATFORMS", None)  # worker needs the axon NC devices
        subprocess.run(
            [sys.executable, os.path.abspath(__file__), "--_worker", in_path, out_path],
            check=True,
            env=env,
        )
        out = np.load(out_path)
        return out["cov"], out["ave"], out["amt"]


if __name__ == "__main__":
    if len(sys.argv) > 1 and sys.argv[1] == "--_worker":
        _worker_main(sys.argv[2:])
    else:
        sys.exit("usage: kernel.py --_worker IN OUT [--trace DIR]")


# revision 18
# speedup vs baseline: 2.6916x; 2.6916x over previous
"""Trainium2 Bass kernel for nn_EstimatorCV (segment_reduce, 8 NeuronCores).

Data-parallel over N: each of the 8 cores reads a 16384-row shard of
`features`, computes per-class partial sums (sum / sumsq / count) via
one-hot matmuls on the TensorEngine, the [C, 2*(A+1)] partials are
all-reduced across the 8 cores, and every core redundantly computes the
tiny EMA combine and writes the [C,A] outputs.

Host-side prep per core:
  - append a ones column to features ([16384, 257]) so the count falls
    out of the same matmuls (column 256 of the partials),
  - reorder rows so each 1 MiB DMA lands contiguous per SBUF partition,
  - transpose labels to [128 partitions, 128 tiles] float32.

The device work runs in a subprocess with a clean JAX environment so the
caller's JAX state (e.g. JAX_PLATFORMS=cpu) cannot break the PJRT path.
"""

import os
import subprocess
import sys
import tempfile

import numpy as np

N_CORES = 8
N, A, C = 131072, 256, 19
RPC = N // N_CORES  # rows per core = 16384
P = 128  # SBUF partitions / rows per matmul tile
AP1 = A + 2  # 258: features + two ones columns (even, for fp32r matmul)
CP = 20  # classes padded to even (fp32r-producing op constraint)
SUB = 8  # row-tiles per DMA group
GROUPS = RPC // (P * SUB)  # 16 groups of ~1MiB per core
GCOLS = SUB * AP1  # 2064 f32 per partition per group
N_TILES = GROUPS * SUB  # 128 row-tiles per core


def _prep_core_inputs(features, labels, core):
    """Build the in_map for one core from the full inputs."""
    sh = features[core * RPC : (core + 1) * RPC]
    f = np.empty((RPC, AP1), np.float32)
    f[:, :A] = sh
    f[:, A:] = 1.0
    # [RPC, AP1] -> [GROUPS, P, SUB*AP1] with rows interleaved so that
    # group g, partition p, subtile j holds row g*P*SUB + j*P + p.
    f = (
        f.reshape(GROUPS, SUB, P, AP1)
        .transpose(0, 2, 1, 3)
        .reshape(GROUPS, P, GCOLS)
    )
    lt = (
        labels[core * RPC : (core + 1) * RPC]
        .reshape(N_TILES, P)
        .T.astype(np.float32)
    )
    return {
        "features": np.ascontiguousarray(f),
        "labels": np.ascontiguousarray(lt),
    }


def _build():
    """Construct the Bass/Tile program (one SPMD NeuronCore view)."""
    from contextlib import ExitStack

    import concourse.mybir as mybir
    import concourse.tile as tile
    from concourse import bacc

    f32 = mybir.dt.float32
    nc = bacc.Bacc(trn_type="TRN2", num_devices=N_CORES)

    feats = nc.dram_tensor("features", [GROUPS, P, GCOLS], f32, kind="ExternalInput")
    labels = nc.dram_tensor("labels", [P, N_TILES], f32, kind="ExternalInput")
    part_out = nc.dram_tensor("part_out", [CP, 2 * AP1], f32, kind="ExternalOutput")

    eq = mybir.AluOpType.is_equal
    f32r = mybir.dt.float32r

    with tile.TileContext(nc) as tc, ExitStack() as ctx:
        const_pool = ctx.enter_context(tc.tile_pool(name="const", bufs=1))
        feat_pool = ctx.enter_context(tc.tile_pool(name="feat", bufs=3))
        featr_pool = ctx.enter_context(tc.tile_pool(name="featr", bufs=2))
        sq_pool = ctx.enter_context(tc.tile_pool(name="sq", bufs=2))
        oh_pool = ctx.enter_context(tc.tile_pool(name="oh", bufs=4))
        psum_pool = ctx.enter_context(tc.tile_pool(name="psum", bufs=1, space="PSUM"))
        tail_pool = ctx.enter_context(tc.tile_pool(name="tail", bufs=1))

        # --- constants / small loads ---
        iota_i = const_pool.tile([P, CP], mybir.dt.int32)
        nc.gpsimd.iota(iota_i[:], pattern=[[1, CP]], base=0, channel_multiplier=0)
        iota_f = const_pool.tile([P, CP], f32)
        nc.vector.tensor_copy(iota_f[:], iota_i[:])

        labels_t = const_pool.tile([P, N_TILES], f32)
        nc.sync.dma_start(labels_t[:], labels.ap())

        # --- main loop: per-class partial sums into PSUM ---
        psum_s = psum_pool.tile([CP, AP1], f32)  # [sum(x) | count]
        psum_q = psum_pool.tile([CP, AP1], f32)  # [sum(x^2) | count]

        for g in range(GROUPS):
            feat = feat_pool.tile([P, GCOLS], f32)
            nc.sync.dma_start(feat[:], feats.ap()[g])
            sq = sq_pool.tile([P, GCOLS], f32r)
            nc.scalar.square(sq[:], feat[:])
            featr = featr_pool.tile([P, GCOLS], f32r)
            nc.vector.tensor_copy(featr[:], feat[:])
            for j in range(SUB):
                it = g * SUB + j
                oh = oh_pool.tile([P, CP], f32r)
                nc.vector.tensor_tensor(
                    out=oh[:],
                    in0=iota_f[:],
                    in1=labels_t[:, it : it + 1].to_broadcast([P, CP]),
                    op=eq,
                )
                first = it == 0
                last = it == N_TILES - 1
                nc.tensor.matmul(
                    psum_s[:],
                    lhsT=oh[:],
                    rhs=featr[:, j * AP1 : (j + 1) * AP1],
                    start=first,
                    stop=last,
                )
                nc.tensor.matmul(
                    psum_q[:],
                    lhsT=oh[:],
                    rhs=sq[:, j * AP1 : (j + 1) * AP1],
                    start=first,
                    stop=last,
                )

        # --- write the per-core partials; reduce + EMA happen on host ---
        part = tail_pool.tile([CP, 2 * AP1], f32)
        nc.vector.tensor_copy(part[:, 0:AP1], psum_s[:])
        nc.vector.tensor_copy(part[:, AP1 : 2 * AP1], psum_q[:])
        nc.sync.dma_start(part_out.ap(), part[:])

    nc.compile()
    return nc


def _host_combine(parts, cov, ave_in, amt):
    """8-way partial reduce + EMA combine (tiny [C,A] math, on host)."""
    red = np.sum(np.asarray(parts, dtype=np.float64), axis=0)[:C]
    s = red[:, 0:A].astype(np.float32)
    cnt = red[:, A].astype(np.float32)
    sq = red[:, AP1 : AP1 + A].astype(np.float32)
    cnt_c = np.maximum(cnt, 1.0)[:, None]
    ave = s / cnt_c
    var = (sq - 2.0 * ave * s + cnt[:, None] * ave * ave) / cnt_c
    denom = cnt + amt
    w = np.where(denom > 0, cnt / np.where(denom > 0, denom, 1.0), 0.0)[:, None]
    additional = w * (1.0 - w) * (ave_in - ave) ** 2
    cov_new = (cov * (1.0 - w) + var * w + additional).astype(np.float32)
    ave_new = (ave_in * (1.0 - w) + ave * w).astype(np.float32)
    amount_new = (amt + cnt).astype(np.float32)
    return cov_new, ave_new, amount_new


def _run_on_device(features, labels, cov, ave, amt, trace=False, tmpdir=None):
    """Shard inputs, compile + execute on 8 NeuronCores, return outputs.

    Must run in an interpreter whose JAX sees the axon NeuronCore devices.
    Returns (cov_new, ave_new, amount_new, exec_time_ns_or_None).
    """
    from concourse.bass_utils import run_bass_kernel_spmd

    nc = _build()
    in_maps = [_prep_core_inputs(features, labels, c) for c in range(N_CORES)]
    res = run_bass_kernel_spmd(
        nc,
        in_maps,
        list(range(N_CORES)),
        trace=trace,
        tmpdir=tmpdir,
    )
    parts = [res.results[c]["part_out"] for c in range(N_CORES)]
    cov_new, ave_new, amount_new = _host_combine(parts, cov, ave, amt)
    return cov_new, ave_new, amount_new, res.exec_time_ns


def _worker_main(argv):
    in_path, out_path = argv[0], argv[1]
    trace = "--trace" in argv
    dat = np.load(in_path)
    cov, ave, amt, exec_ns = _run_on_device(
        dat["features"],
        dat["labels"],
        dat["cov"],
        dat["ave"],
        dat["amt"],
        trace=trace,
        tmpdir=(argv[argv.index("--trace") + 1] if trace else None),
    )
    np.savez(
        out_path,
        cov=cov,
        ave=ave,
        amt=amt,
        exec_ns=np.int64(exec_ns if exec_ns is not None else -1),
    )


def kernel(features, labels, CoVariance, Ave, Amount):
    features = np.ascontiguousarray(np.asarray(features), dtype=np.float32)
    labels = np.ascontiguousarray(np.asarray(labels), dtype=np.int64)
    cov = np.ascontiguousarray(np.asarray(CoVariance), dtype=np.float32)
    ave = np.ascontiguousarray(np.asarray(Ave), dtype=np.float32)
    amt = np.ascontiguousarray(np.asarray(Amount), dtype=np.float32)

    with tempfile.TemporaryDirectory() as td:
        in_path = os.path.join(td, "in.npz")
        out_path = os.path.join(td, "out.npz")
        np.savez(in_path, features=features, labels=labels, cov=cov, ave=ave, amt=amt)
        env = dict(os.environ)
        env.pop("JAX_PLATFORMS", None)  # worker needs the axon NC devices
        subprocess.run(
            [sys.executable, os.path.abspath(__file__), "--_worker", in_path, out_path],
            check=True,
            env=env,
        )
        out = np.load(out_path)
        return out["cov"], out["ave"], out["amt"]


if __name__ == "__main__":
    if len(sys.argv) > 1 and sys.argv[1] == "--_worker":
        _worker_main(sys.argv[2:])
    else:
        sys.exit("usage: kernel.py --_worker IN OUT [--trace DIR]")
